# revision 1
# baseline (speedup 1.0000x reference)
"""CharTransformer forward on 8 TRN2 NeuronCores — v3 (v2 + ACT->DVE copy rebalance).

Sharding: DP over batch (2 groups of 4 cores). Within a group:
- Residual stream / LN / FFN are sequence-parallel (512 tokens per core).
- Attention is head-parallel: each core owns 4 of the 16 heads for the FULL
  2048-token sample. Per layer: LN1 locally, AllGather xn within the group,
  each core projects Q/K/V for its 4 heads over all T, runs causally-skipped
  attention (uniform across cores since every core sees all queries), computes
  its Wo partial for all tokens, and a ReduceScatter hands each core the
  summed attention update for its own 512 tokens.

Causal structure is compile-time uniform: for query chunk tq (512 wide) only
key chunks j < 4*(tq+1) are touched; the 4 diagonal blocks use one constant
[128,128] triangular mask. This halves score/AV work and removes the 4x
redundant K/V of the v1 kernel.

All matmul moving operands are bf16 (fp32 moving costs 4 cycles/col on PE).
Residual fp32, PSUM accum fp32.
"""

import numpy as np
import ml_dtypes

import concourse.bass as bass
import concourse.bacc as bacc
import concourse.mybir as mybir
import concourse.tile as tile
from concourse.bass_utils import run_bass_kernel_spmd

B, T, E, H, D, L, F, V = 2, 2048, 1024, 16, 64, 6, 4096, 256
NT = 512                 # tokens per core (residual stream)
EC = E // 128            # 8 feature chunks
NKC = T // 128           # 16 key chunks
FB = F // 128            # 32 ffn blocks
HL = 4                   # heads per core
HD = HL * D              # 256 head dims per core
PC = 2                   # pair chunks (2 heads of 64 = 128 partitions)
N_CORES = 8
GROUPS = [[0, 1, 2, 3], [4, 5, 6, 7]]
F32 = mybir.dt.float32
BF16 = mybir.dt.bfloat16
AF = mybir.ActivationFunctionType
ALU = mybir.AluOpType
BF = ml_dtypes.bfloat16

_BUILD_CACHE = {}


def _build(n_layers=L, stage=None):
    nc = bacc.Bacc("TRN2", target_bir_lowering=False, debug=False, num_devices=N_CORES)

    # ---- kernel I/O (per-core slices prepared host-side) ----
    x0t_d = nc.dram_tensor("x0t", [E, NT], F32, kind="ExternalInput")
    m128_d = nc.dram_tensor("m128", [128, 128], BF16, kind="ExternalInput")
    wq_d = nc.dram_tensor("wq", [L, E, HD], BF16, kind="ExternalInput")
    wk_d = nc.dram_tensor("wk", [L, E, HD], BF16, kind="ExternalInput")
    wv_d = nc.dram_tensor("wv", [L, E, HD], BF16, kind="ExternalInput")
    wo_d = nc.dram_tensor("wo", [L, HD, E], BF16, kind="ExternalInput")
    w1_d = nc.dram_tensor("w1", [L, E, F], BF16, kind="ExternalInput")
    w2_d = nc.dram_tensor("w2", [L, F, E], BF16, kind="ExternalInput")
    lmw_d = nc.dram_tensor("lmw", [E, V], BF16, kind="ExternalInput")
    bo_d = nc.dram_tensor("bo", [L, E], F32, kind="ExternalInput")
    b1_d = nc.dram_tensor("b1", [L, F], F32, kind="ExternalInput")
    b2_d = nc.dram_tensor("b2", [L, E], F32, kind="ExternalInput")
    ln1g_d = nc.dram_tensor("ln1g", [L, E], F32, kind="ExternalInput")
    ln1b_d = nc.dram_tensor("ln1b", [L, E], F32, kind="ExternalInput")
    ln2g_d = nc.dram_tensor("ln2g", [L, E], F32, kind="ExternalInput")
    ln2b_d = nc.dram_tensor("ln2b", [L, E], F32, kind="ExternalInput")
    lnfg_d = nc.dram_tensor("lnfg", [E], F32, kind="ExternalInput")
    lnfb_d = nc.dram_tensor("lnfb", [E], F32, kind="ExternalInput")
    lmb_d = nc.dram_tensor("lmb", [V], F32, kind="ExternalInput")
    out_d = nc.dram_tensor("outT", [V, NT], F32, kind="ExternalOutput")

    # collective bounce buffers
    xnag_d = nc.dram_tensor("xnag", [E * NT], BF16)          # own xn, [e, t]
    xnall_d = nc.dram_tensor("xnall", [4, E * NT], BF16)     # gathered xn
    xps_d = nc.dram_tensor("xps", [4, E * NT], BF16)         # Wo partials, token-block major
    xrs_d = nc.dram_tensor("xrs", [E * NT], BF16)            # reduced own-token update

    def dap(t, offset, dims):
        return bass.AP(t, offset, dims)

    with tile.TileContext(nc, num_cores=N_CORES) as tc:
        with (
            tc.tile_pool(name="big", bufs=1) as bigp,
            tc.tile_pool(name="wt", bufs=3) as wtp,
            tc.tile_pool(name="ws", bufs=1) as wsp,
            tc.tile_pool(name="xnr", bufs=2) as xnrp,
            tc.tile_pool(name="sc", bufs=2) as scp,
            tc.tile_pool(name="sc3", bufs=3) as sc3p,
            tc.tile_pool(name="psum", bufs=2, space="PSUM") as psp,
            tc.tile_pool(name="psumo", bufs=4, space="PSUM") as psO,
        ):
            # ---- constants ----
            ones_bf = bigp.tile([128, 1], BF16, tag="ones_bf")
            one1 = bigp.tile([1, 128], F32, tag="one1")
            one1b = bigp.tile([1, 128], BF16, tag="one1b")
            epsT = bigp.tile([1, 1], F32, tag="epsT")
            qsc = bigp.tile([128, 1], F32, tag="qsc")
            m128 = bigp.tile([128, 128], BF16, tag="m128")
            nc.vector.memset(ones_bf[:], 1.0)
            nc.vector.memset(one1[:], 1.0)
            nc.vector.memset(one1b[:], 1.0)
            nc.vector.memset(epsT[:], 1e-5)
            nc.vector.memset(qsc[:], float(D) ** -0.5)
            nc.sync.dma_start(m128[:], m128_d[:])

            # ---- persistent activations ----
            xT = bigp.tile([128, EC * NT], F32, tag="xT")          # residual, [e, t]
            Q_sb = bigp.tile([128, PC * T], BF16, tag="Q")         # [pair-dim, t]
            K_sb = bigp.tile([128, PC * T], BF16, tag="K")
            V_sb = bigp.tile([128, NKC * (HL * 65)], BF16, tag="V")  # [s-chunk, (j, h, d|1)]
            OTn2 = bigp.tile([128, PC * T], BF16, tag="OTn2")      # attn out, pair-packed
            h_sb = bigp.tile([128, FB * NT], BF16, tag="hsb")      # ffn hidden

            nc.vector.memset(V_sb[:], 1.0)  # bakes the 65th (denominator) column
            nc.sync.dma_start(
                xT[:], dap(x0t_d, 0, [[NT, 128], [128 * NT, EC], [1, NT]]))

            def layer_norm(g_sl, b_sl, out_bf):
                """out_bf[128, EC*NT] bf16 = LN(xT) * g + b."""
                st1 = psp.tile([1, NT], F32, tag="mm")
                st2 = psp.tile([1, NT], F32, tag="mm")
                for k in range(EC):
                    col = slice(k * NT, (k + 1) * NT)
                    xb = sc3p.tile([128, NT], BF16, tag="lnb")
                    x2 = sc3p.tile([128, NT], BF16, tag="lnb")
                    nc.vector.tensor_copy(xb[:], xT[:, col])
                    nc.scalar.activation(x2[:], xT[:, col], AF.Square)
                    nc.tensor.matmul(st1[:], ones_bf[:], xb[:],
                                     start=(k == 0), stop=(k == EC - 1))
                    nc.tensor.matmul(st2[:], ones_bf[:], x2[:],
                                     start=(k == 0), stop=(k == EC - 1))
                rowA = scp.tile([1, NT], F32, tag="lnA")   # mean -> -mean*rstd
                rowB = scp.tile([1, NT], F32, tag="lnB")   # msq -> var -> rstd
                nc.vector.tensor_scalar_mul(rowA[:], st1[:], 1.0 / E)
                nc.scalar.activation(rowB[:], rowA[:], AF.Square)
                nc.vector.scalar_tensor_tensor(
                    rowB[:], st2[:], 1.0 / E, rowB[:], op0=ALU.mult, op1=ALU.subtract)
                nc.scalar.activation(rowB[:], rowB[:], AF.Sqrt, bias=epsT[:])
                nc.vector.reciprocal(rowB[:], rowB[:])
                nc.vector.scalar_tensor_tensor(
                    rowA[:], rowA[:], -1.0, rowB[:], op0=ALU.mult, op1=ALU.mult)
                Ab = psp.tile([128, NT], F32, tag="bc")
                Cb = psp.tile([128, NT], F32, tag="bc")
                nc.tensor.matmul(Ab[:], one1[:], rowB[:], start=True, stop=True)
                nc.tensor.matmul(Cb[:], one1[:], rowA[:], start=True, stop=True)
                for k in range(EC):
                    col = slice(k * NT, (k + 1) * NT)
                    tmp = sc3p.tile([128, NT], F32, tag="lnt")
                    nc.vector.tensor_tensor(tmp[:], xT[:, col], Ab[:], op=ALU.mult)
                    nc.vector.tensor_tensor(tmp[:], tmp[:], Cb[:], op=ALU.add)
                    nc.vector.tensor_scalar(
                        out_bf[:, col], tmp[:], g_sl[:, k:k + 1], b_sl[:, k:k + 1],
                        op0=ALU.mult, op1=ALU.add)

            def tap(src_ap, parts=128):
                for tvb in range(2):
                    tp = sc3p.tile([128, NT], F32, tag="lnt")
                    nc.vector.memset(tp[:], 0.0)
                    nc.scalar.copy(tp[0:parts, :],
                                   src_ap[0:parts, tvb * NT:(tvb + 1) * NT])
                    nc.sync.dma_start(out_d[tvb * 128:(tvb + 1) * 128, :], tp[:])

            for l in range(n_layers):
                # ---- per-layer parameter rows ----
                l1g = scp.tile([128, EC], F32, tag="p_l1g")
                l1b = scp.tile([128, EC], F32, tag="p_l1b")
                l2g = scp.tile([128, EC], F32, tag="p_l2g")
                l2b = scp.tile([128, EC], F32, tag="p_l2b")
                bo_s = scp.tile([128, EC], F32, tag="p_bo")
                b2_s = scp.tile([128, EC], F32, tag="p_b2")
                b1_s = scp.tile([128, FB], F32, tag="p_b1")
                rowap = [[1, 128], [128, EC]]
                nc.sync.dma_start(l1g[:], dap(ln1g_d, l * E, rowap))
                nc.sync.dma_start(l1b[:], dap(ln1b_d, l * E, rowap))
                nc.sync.dma_start(l2g[:], dap(ln2g_d, l * E, rowap))
                nc.sync.dma_start(l2b[:], dap(ln2b_d, l * E, rowap))
                nc.sync.dma_start(bo_s[:], dap(bo_d, l * E, rowap))
                nc.sync.dma_start(b2_s[:], dap(b2_d, l * E, rowap))
                nc.sync.dma_start(b1_s[:], dap(b1_d, l * F, [[1, 128], [128, FB]]))

                # ---- per-layer weights (double-buffered across layers) ----
                wq_sb = wsp.tile([128, PC * EC * 128], BF16, tag="wq")
                wk_sb = wsp.tile([128, PC * EC * 128], BF16, tag="wk")
                wv_sb = wsp.tile([128, EC * HD], BF16, tag="wv")
                wo_sb = wsp.tile([128, PC * E], BF16, tag="wo")
                for pc in range(PC):
                    wcol = slice(pc * EC * 128, (pc + 1) * EC * 128)
                    nc.sync.dma_start(
                        wq_sb[:, wcol],
                        dap(wq_d, l * E * HD + pc * 128,
                            [[HD, 128], [128 * HD, EC], [1, 128]]))
                    nc.sync.dma_start(
                        wk_sb[:, wcol],
                        dap(wk_d, l * E * HD + pc * 128,
                            [[HD, 128], [128 * HD, EC], [1, 128]]))
                nc.sync.dma_start(
                    wv_sb[:], dap(wv_d, l * E * HD,
                                  [[HD, 128], [128 * HD, EC], [1, HD]]))
                nc.sync.dma_start(
                    wo_sb[:], dap(wo_d, l * HD * E,
                                  [[E, 128], [128 * E, PC], [1, E]]))

                # ---- LN1 -> xn ----
                xn = bigp.tile([128, EC * NT], BF16, tag="xn")
                layer_norm(l1g, l1b, xn)
                if stage == "ln":
                    tap(xn)
                    break

                # ---- AllGather xn within the 4-core group ----
                nc.sync.dma_start(
                    dap(xnag_d, 0, [[NT, 128], [128 * NT, EC], [1, NT]]), xn[:])
                nc.gpsimd.collective_compute(
                    "AllGather", ALU.bypass, replica_groups=GROUPS,
                    ins=[xnag_d[:].opt()], outs=[xnall_d[:].opt()])

                # ---- Q/K/V for my 4 heads over the full sample ----
                for tq in range(4):
                    xnrb = xnrp.tile([128, EC * NT], BF16, tag="xnrb")
                    nc.sync.dma_start(
                        xnrb[:], dap(xnall_d, tq * E * NT,
                                     [[NT, 128], [128 * NT, EC], [1, NT]]))
                    for pc in range(PC):
                        qp = psp.tile([128, NT], F32, tag="mm")
                        for ec in range(EC):
                            nc.tensor.matmul(
                                qp[:], wq_sb[:, (pc * EC + ec) * 128:(pc * EC + ec + 1) * 128],
                                xnrb[:, ec * NT:(ec + 1) * NT],
                                start=(ec == 0), stop=(ec == EC - 1))
                        nc.vector.tensor_copy(
                            Q_sb[:, pc * T + tq * NT:pc * T + (tq + 1) * NT], qp[:])
                        kp = psp.tile([128, NT], F32, tag="mm")
                        for ec in range(EC):
                            nc.tensor.matmul(
                                kp[:], wk_sb[:, (pc * EC + ec) * 128:(pc * EC + ec + 1) * 128],
                                xnrb[:, ec * NT:(ec + 1) * NT],
                                start=(ec == 0), stop=(ec == EC - 1))
                        nc.vector.tensor_copy(
                            K_sb[:, pc * T + tq * NT:pc * T + (tq + 1) * NT], kp[:])
                    for jq in range(4):
                        j = tq * 4 + jq
                        vp = psp.tile([128, HD], F32, tag="mm")
                        for ec in range(EC):
                            nc.tensor.matmul(
                                vp[:], xnrb[:, ec * NT + jq * 128:ec * NT + (jq + 1) * 128],
                                wv_sb[:, ec * HD:(ec + 1) * HD],
                                start=(ec == 0), stop=(ec == EC - 1))
                        for hl in range(HL):
                            nc.vector.tensor_copy(
                                V_sb[:, j * (HL * 65) + hl * 65:
                                     j * (HL * 65) + hl * 65 + 64],
                                vp[:, hl * 64:(hl + 1) * 64])
                if stage == "qkv":
                    tap(Q_sb)
                    break

                # ---- causal attention, my heads, all queries ----
                for hl in range(HL):
                    pc, ho = hl // 2, 64 * (hl % 2)
                    for tq in range(4):
                        otp = psO.tile([65, NT], F32, tag="otp",
                                       name=f"otp_{l}_{hl}_{tq}")
                        nj = 4 * (tq + 1)
                        for j in range(nj):
                            k = j - 4 * tq
                            cols = NT - max(0, 128 * k)
                            toff = NT * (tq + 1) - cols       # absolute t start
                            S = psp.tile([128, NT], F32, tag="mm")
                            nc.tensor.matmul(
                                S[:, 0:cols],
                                K_sb[ho:ho + 64, pc * T + j * 128:pc * T + (j + 1) * 128],
                                Q_sb[ho:ho + 64, pc * T + toff:pc * T + toff + cols],
                                start=True, stop=True)
                            P = sc3p.tile([128, NT], BF16, tag="P")
                            nc.scalar.activation(P[:, 0:cols], S[:, 0:cols],
                                                 AF.Exp, scale=qsc[:])
                            if k >= 0:
                                nc.vector.tensor_tensor(
                                    P[:, 0:128], P[:, 0:128], m128[:], op=ALU.mult)
                            nc.tensor.matmul(
                                otp[:, NT - cols:NT],
                                V_sb[:, j * (HL * 65) + hl * 65:
                                     j * (HL * 65) + (hl + 1) * 65],
                                P[:, 0:cols],
                                start=(j == 0), stop=(j == nj - 1))
                        rlf = scp.tile([1, NT], F32, tag="rlf")
                        rlb16 = scp.tile([1, NT], BF16, tag="rlb16")
                        nc.vector.reciprocal(rlf[:], otp[64:65, :])
                        nc.vector.tensor_copy(rlb16[:], rlf[:])
                        rlb = psp.tile([128, NT], F32, tag="bc")
                        nc.tensor.matmul(rlb[:], one1b[:], rlb16[:],
                                         start=True, stop=True)
                        rlb_s = sc3p.tile([64, NT], F32, tag="rlbs")
                        nc.scalar.copy(rlb_s[:], rlb[0:64, :])
                        ocol = slice(pc * T + tq * NT, pc * T + (tq + 1) * NT)
                        if ho == 0:
                            nc.vector.tensor_tensor(
                                OTn2[0:64, ocol], otp[0:64, :], rlb_s[:],
                                op=ALU.mult)
                        else:
                            nrm = sc3p.tile([64, NT], BF16, tag="nrm")
                            nc.vector.tensor_tensor(
                                nrm[:], otp[0:64, :], rlb_s[:], op=ALU.mult)
                            nc.sync.dma_start(OTn2[64:128, ocol], nrm[:])
                if stage == "attn":
                    tap(OTn2)
                    break

                # ---- Wo partials for all tokens -> DRAM (token-block major) ----
                for tb in range(4):
                    for eb in range(EC):
                        xa = psp.tile([128, NT], F32, tag="mm")
                        for pc in range(PC):
                            nc.tensor.matmul(
                                xa[:], wo_sb[:, pc * E + eb * 128:pc * E + (eb + 1) * 128],
                                OTn2[:, pc * T + tb * NT:pc * T + (tb + 1) * NT],
                                start=(pc == 0), stop=(pc == PC - 1))
                        xab = sc3p.tile([128, NT], BF16, tag="xab")
                        nc.vector.tensor_copy(xab[:], xa[:])
                        nc.sync.dma_start(
                            dap(xps_d, tb * E * NT + eb * 128 * NT,
                                [[NT, 128], [1, NT]]),
                            xab[:])
                nc.gpsimd.collective_compute(
                    "ReduceScatter", ALU.add, replica_groups=GROUPS,
                    ins=[xps_d[:].opt()], outs=[xrs_d[:].opt()])

                # ---- residual += attn update + bo ----
                xru = wsp.tile([128, EC * NT], BF16, tag="xru")
                nc.sync.dma_start(
                    xru[:], dap(xrs_d, 0, [[NT, 128], [128 * NT, EC], [1, NT]]))
                for eb in range(EC):
                    col = slice(eb * NT, (eb + 1) * NT)
                    nc.vector.scalar_tensor_tensor(
                        xT[:, col], xru[:, col], bo_s[:, eb:eb + 1], xT[:, col],
                        op0=ALU.add, op1=ALU.add)
                if stage == "wo":
                    tap(xT)
                    break

                # ---- LN2 -> xn2 ----
                xn2 = bigp.tile([128, EC * NT], BF16, tag="xn")
                layer_norm(l2g, l2b, xn2)

                # ---- FFN (single pass, weights streamed) ----
                for fb in range(FB):
                    w1t = wtp.tile([128, EC * 128], BF16, tag="w1t")
                    nc.sync.dma_start(
                        w1t[:], dap(w1_d, l * E * F + fb * 128,
                                    [[F, 128], [128 * F, EC], [1, 128]]))
                    hp = psp.tile([128, NT], F32, tag="mm")
                    for ec in range(EC):
                        nc.tensor.matmul(
                            hp[:], w1t[:, ec * 128:(ec + 1) * 128],
                            xn2[:, ec * NT:(ec + 1) * NT],
                            start=(ec == 0), stop=(ec == EC - 1))
                    nc.scalar.activation(
                        h_sb[:, fb * NT:(fb + 1) * NT], hp[:], AF.Relu,
                        bias=b1_s[:, fb:fb + 1])
                for eb in range(EC):
                    w2t = wtp.tile([128, FB * 128], BF16, tag="w2t")
                    nc.sync.dma_start(
                        w2t[:], dap(w2_d, l * F * E + eb * 128,
                                    [[E, 128], [128 * E, FB], [1, 128]]))
                    yp = psp.tile([128, NT], F32, tag="mm")
                    for fc in range(FB):
                        nc.tensor.matmul(
                            yp[:], w2t[:, fc * 128:(fc + 1) * 128],
                            h_sb[:, fc * NT:(fc + 1) * NT],
                            start=(fc == 0), stop=(fc == FB - 1))
                    col = slice(eb * NT, (eb + 1) * NT)
                    nc.vector.scalar_tensor_tensor(
                        xT[:, col], yp[:], b2_s[:, eb:eb + 1], xT[:, col],
                        op0=ALU.add, op1=ALU.add)

            # ---- final LN + lm head ----
            lfg = scp.tile([128, EC], F32, tag="p_l1g")
            lfb = scp.tile([128, EC], F32, tag="p_l1b")
            lmb_s = scp.tile([128, 2], F32, tag="p_lmb")
            nc.sync.dma_start(lfg[:], dap(lnfg_d, 0, [[1, 128], [128, EC]]))
            nc.sync.dma_start(lfb[:], dap(lnfb_d, 0, [[1, 128], [128, EC]]))
            nc.sync.dma_start(lmb_s[:], dap(lmb_d, 0, [[1, 128], [128, 2]]))
            xnf = bigp.tile([128, EC * NT], BF16, tag="xn")
            layer_norm(lfg, lfb, xnf)
            for vb in range(2):
                lmwt = scp.tile([128, EC * 128], BF16, tag="lmwt")
                nc.sync.dma_start(
                    lmwt[:], dap(lmw_d, vb * 128, [[V, 128], [128 * V, EC], [1, 128]]))
                lp = psp.tile([128, NT], F32, tag="mm")
                for ec in range(EC):
                    nc.tensor.matmul(
                        lp[:], lmwt[:, ec * 128:(ec + 1) * 128],
                        xnf[:, ec * NT:(ec + 1) * NT],
                        start=(ec == 0), stop=(ec == EC - 1))
                lg = sc3p.tile([128, NT], F32, tag="lnt")
                nc.vector.tensor_scalar(
                    lg[:], lp[:], lmb_s[:, vb:vb + 1], None, op0=ALU.add)
                nc.sync.dma_start(out_d[vb * 128:(vb + 1) * 128, :], lg[:])

    nc.compile()
    return nc


def _get_nc(n_layers=L, stage=None):
    key = (n_layers, stage)
    if key not in _BUILD_CACHE:
        _BUILD_CACHE[key] = _build(n_layers, stage)
    return _BUILD_CACHE[key]


def _prep_inputs(idx, embed, pos_embed, Wq, Wk, Wv, Wo, bo, W1, b1, W2, b2,
                 ln1_g, ln1_b, ln2_g, ln2_b, lnf_g, lnf_b, lmW, lmb):
    idx = np.asarray(idx)
    embed = np.asarray(embed, np.float32)
    pos = np.asarray(pos_embed, np.float32)
    x0 = embed[idx] + pos[None, :T]                       # (B, T, E) f32

    # [L, E, H*D] with (h, d) minor — per-core col slices are head slices
    wq_h = np.ascontiguousarray(
        np.transpose(np.asarray(Wq, np.float32), (0, 2, 1, 3)).reshape(L, E, E)
    ).astype(BF)
    wk_h = np.ascontiguousarray(
        np.transpose(np.asarray(Wk, np.float32), (0, 2, 1, 3)).reshape(L, E, E)
    ).astype(BF)
    wv_h = np.ascontiguousarray(
        np.transpose(np.asarray(Wv, np.float32), (0, 2, 1, 3)).reshape(L, E, E)
    ).astype(BF)
    wo_h = np.asarray(Wo, np.float32).astype(BF)          # [L, E(=h,d), E]
    w1_h = np.asarray(W1, np.float32).astype(BF)
    w2_h = np.asarray(W2, np.float32).astype(BF)
    lmw_h = np.asarray(lmW, np.float32).astype(BF)

    r = np.arange(128)
    m128 = (r[:, None] <= r[None, :]).astype(BF)          # keep key r <= query c

    shared = {
        "w1": w1_h, "w2": w2_h, "lmw": lmw_h, "m128": m128,
        "bo": np.asarray(bo, np.float32), "b1": np.asarray(b1, np.float32),
        "b2": np.asarray(b2, np.float32),
        "ln1g": np.asarray(ln1_g, np.float32), "ln1b": np.asarray(ln1_b, np.float32),
        "ln2g": np.asarray(ln2_g, np.float32), "ln2b": np.asarray(ln2_b, np.float32),
        "lnfg": np.asarray(lnf_g, np.float32), "lnfb": np.asarray(lnf_b, np.float32),
        "lmb": np.asarray(lmb, np.float32),
    }

    in_maps = []
    for c in range(N_CORES):
        b, g = c // 4, c % 4
        hs = slice(g * HD, (g + 1) * HD)
        m = dict(shared)
        m["x0t"] = np.ascontiguousarray(x0[b, g * NT:(g + 1) * NT, :].T)
        m["wq"] = np.ascontiguousarray(wq_h[:, :, hs])
        m["wk"] = np.ascontiguousarray(wk_h[:, :, hs])
        m["wv"] = np.ascontiguousarray(wv_h[:, :, hs])
        m["wo"] = np.ascontiguousarray(wo_h[:, hs, :])
        in_maps.append(m)
    return in_maps


def kernel(idx, embed, pos_embed, Wq, Wk, Wv, Wo, bo, W1, b1, W2, b2,
           ln1_g, ln1_b, ln2_g, ln2_b, lnf_g, lnf_b, lmW, lmb,
           _n_layers=L, _stage=None, _sim=False):
    in_maps = _prep_inputs(idx, embed, pos_embed, Wq, Wk, Wv, Wo, bo, W1, b1,
                           W2, b2, ln1_g, ln1_b, ln2_g, ln2_b, lnf_g, lnf_b,
                           lmW, lmb)
    nc = _get_nc(_n_layers, _stage)

    if _sim:
        from concourse.bass_interp import MultiCoreSim
        sim = MultiCoreSim(nc, num_cores=N_CORES, num_workers=N_CORES)
        for cid, core_sim in sim.cores.items():
            for name, val in in_maps[cid].items():
                core_sim.tensor(name)[:] = val
        sim.simulate()
        results = [{"outT": np.asarray(sim.cores[c].tensor("outT"))}
                   for c in range(N_CORES)]
        global _LAST_SIM_TIME
        _LAST_SIM_TIME = sim.global_time
        print(f"[sim] global_time = {sim.global_time} ns")
    else:
        res = run_bass_kernel_spmd(nc, in_maps, list(range(N_CORES)))
        global _LAST_RES
        _LAST_RES = res
        results = res.results

    out = np.empty((B, T, V), np.float32)
    for c in range(N_CORES):
        b, g = c // 4, c % 4
        out[b, g * NT:(g + 1) * NT, :] = results[c]["outT"].T
    return out

